# revision 27
# baseline (speedup 1.0000x reference)
"""Trainium2 Bass kernel for the BidirectionalAttentionModule problem.

Sharding: data-parallel over batch — 16 batches across 8 NeuronCores
(2 per core), weights replicated.

Device computes the two large outputs per batch b and layer i
(D=256, LC=512, LP=2048):
    Hc  = tanh(comp @ W_hc[i])                      [LC, D]
    Hp  = tanh(prot @ W_hp[i])                      [LP, D]
    At  = tanh(comp @ U[i] @ prot.T)                [LC, LP]   (unmasked)
    cc  = cm * (At  @ (pm * tanh(prot @ W_p2c[i]))) [LC, D]
    pc  = pm * (At.T @ (cm * tanh(comp @ W_c2p[i]))) [LP, D]
    comp_layerwise[b,:,512i:512i+512] = [Hc | cc]
    prot_layerwise[b,:,512i:512i+512] = [Hp | pc]
(the reference's masked A = At * cm[:,None] * pm[None,:] appears only
inside the two cross matmuls, so the mask folds into row/col scalings).

The softmax-pooling tail (comp_final / prot_final, <0.3% of FLOPs) is
computed on host in fp32 numpy from the returned layerwise tensors.

Matmuls run as float32r (single-pass reduced-precision fp32, ~1.2e-4
rel err) — 4x faster than exact fp32 on the PE.

The layer loop is software-pipelined: each layer's PE-dense back half
(S^T recompute + the two cross matmuls) is emitted interleaved with the
next layer's ACT-dense front half (projections + S scores) so neither
engine starves.
"""

import sys

if "/opt/trn_rl_repo" not in sys.path:
    sys.path.insert(0, "/opt/trn_rl_repo")

import numpy as np

import concourse.bass as bass
import concourse.mybir as mybir
import concourse.tile as tile
from concourse import bacc
from concourse.bass_utils import run_bass_kernel_spmd
from concourse.masks import make_identity

B, LC, LP, D, L = 16, 512, 2048, 256, 4
NCORES = 8
BPC = B // NCORES   # batches per core
NCK = LC // 128     # 4 comp chunks
NPK = LP // 128     # 16 prot chunks
F32 = mybir.dt.float32
F32R = mybir.dt.float32r
TANH = mybir.ActivationFunctionType.Tanh

_NC_CACHE = {}


class _Env:
    pass


class _LState:
    """Per-(batch, layer) tiles in flight."""
    def __init__(self, i):
        self.i = i
        self.ttT = None
        self.vcms = []
        self.vpms = []
        self.ahalves = {}   # (ck, n2) -> [128, 1024] tile of At rows ck
        self.atTs = []


class _BState:
    """Per-batch tiles/APs."""
    pass


def _gen_pre(e, bs, bi):
    """Feature loads + on-chip transposes for one batch, unit-at-a-time."""
    nc = e.nc
    # loads go out on the ACT HWDGE ring so they don't queue behind the
    # previous batch's store stream on the SP ring
    craw = e.p_raw.tile([128, NCK, D], F32, tag="craw")
    nc.scalar.dma_start(craw[:, :, :],
                        e.comp[bi].rearrange("(c p) d -> p c d", p=128))
    prot_r = e.prot[bi].rearrange("(c p) d -> p c d", p=128)
    praws = []
    for p in range(4):
        pr = e.p_raw.tile([128, 4, D], F32, tag="praw", bufs=4, name=f"praw{p}")
        nc.scalar.dma_start(pr[:, :, :], prot_r[:, 4 * p:4 * p + 4, :])
        praws.append(pr)
    bs.cmtile = e.p_mask.tile([128, NCK], F32, tag="cmt")
    bs.pmtile = e.p_mask.tile([128, NPK], F32, tag="pmt")
    nc.scalar.dma_start(bs.cmtile[:, :], e.cmt[bi])
    nc.scalar.dma_start(bs.pmtile[:, :], e.pmt[bi])
    bs.ocb = e.oc[bi].rearrange("(c p) f -> p c f", p=128)   # [128, 4, 2048]
    bs.opb = e.op[bi].rearrange("(c p) f -> p c f", p=128)   # [128, 16, 2048]

    bs.compT = e.p_feat.tile([128, 2, LC], F32R, tag="compT")
    bs.protT = e.p_feat.tile([128, 2, LP], F32R, tag="protT")
    for k in range(2):
        psx = e.ps_b.tile([128, 2, 512], F32, tag="psb")
        for ck in range(NCK):
            nc.tensor.transpose(
                psx[:, 0, ck * 128:(ck + 1) * 128],
                craw[:, ck, k * 128:(k + 1) * 128],
                e.ident[:, :],
            )
        nc.vector.tensor_copy(bs.compT[:, k, :], psx[:, 0, :])
    yield
    for p in range(4):
        for k in range(2):
            psx = e.ps_b.tile([128, 2, 512], F32, tag="psb")
            for jj in range(4):
                nc.tensor.transpose(
                    psx[:, 0, jj * 128:(jj + 1) * 128],
                    praws[p][:, jj, k * 128:(k + 1) * 128],
                    e.ident[:, :],
                )
            nc.vector.tensor_copy(
                bs.protT[:, k, p * 512:(p + 1) * 512], psx[:, 0, :]
            )
        yield


def _gen_front(e, bs, st):
    """Weights+TT, comp projections, prot projections, S scores.
    Yields after each pipelineable unit."""
    nc = e.nc
    i = st.i

    # --- weights + T^T ---
    utile = e.p_w.tile([128, 2, 2, 128], F32R, tag="u")
    wct = e.p_w.tile([128, 2, 512], F32R, tag="wc")
    wpt = e.p_w.tile([128, 2, 512], F32R, tag="wp")
    nc.gpsimd.dma_start(utile[:, :, :, :], e.usb[i])
    nc.gpsimd.dma_start(wct[:, :, :], e.wcd[i])
    nc.gpsimd.dma_start(wpt[:, :, :], e.wpd[i])
    if e.with_bias:
        bct = e.p_w.tile([1, 512], F32R, tag="bc")
        bpt = e.p_w.tile([1, 512], F32R, tag="bp")
        nc.gpsimd.dma_start(bct[:, :], e.bcd[i])
        nc.gpsimd.dma_start(bpt[:, :], e.bpd[i])
    ttps = e.ps_b.tile([128, 2, 512], F32, tag="psb")
    for m in range(2):
        for k in range(2):
            nc.tensor.matmul(
                ttps[:, m, :], utile[:, k, m, :], bs.compT[:, k, :],
                start=(k == 0), stop=(k == 1),
            )
    st.ttT = e.p_tt.tile([128, 2, LC], F32R, tag="ttT")
    nc.vector.tensor_copy(st.ttT[:, :, :], ttps[:, :, :])
    yield

    # --- comp projections: [Hc | Vc] ---
    for cg in range(2):
        ppc = e.ps_b.tile([128, 2, 512], F32, tag="psb")
        for cc in range(2):
            ck = 2 * cg + cc
            if e.with_bias:
                nc.tensor.matmul(ppc[:, cc, :], e.ones[:, :], bct[:, :],
                                 start=True, stop=False)
            for k in range(2):
                nc.tensor.matmul(
                    ppc[:, cc, :],
                    bs.compT[:, k, ck * 128:(ck + 1) * 128],
                    wct[:, k, :],
                    start=(k == 0 and not e.with_bias), stop=(k == 1),
                )
        hcvc = e.p_hv.tile([128, 2, 512], F32, tag="hcvc")
        nc.scalar.activation(hcvc[:, :, :], ppc[:, :, :], TANH)
        vcm = e.p_vm.tile([128, 2, 256], F32R, tag="vcm")
        for cc in range(2):
            ck = 2 * cg + cc
            nc.vector.tensor_scalar_mul(
                vcm[:, cc, :], hcvc[:, cc, 256:512], bs.cmtile[:, ck:ck + 1]
            )
        st.vcms.append(vcm)
        nc.sync.dma_start(
            bs.ocb[:, 2 * cg:2 * cg + 2, i * 512:i * 512 + 256],
            hcvc[:, :, 0:256],
        )
        yield

    # --- prot projections: [Hp | Vp] ---
    for g in range(8):
        ppp = e.ps_b.tile([128, 2, 512], F32, tag="psb")
        for jj in range(2):
            j = 2 * g + jj
            if e.with_bias:
                nc.tensor.matmul(ppp[:, jj, :], e.ones[:, :], bpt[:, :],
                                 start=True, stop=False)
            for k in range(2):
                nc.tensor.matmul(
                    ppp[:, jj, :],
                    bs.protT[:, k, j * 128:(j + 1) * 128],
                    wpt[:, k, :],
                    start=(k == 0 and not e.with_bias), stop=(k == 1),
                )
        hpvp = e.p_hv.tile([128, 2, 512], F32, tag="hpvp", bufs=3)
        nc.scalar.activation(hpvp[:, :, :], ppp[:, :, :], TANH)
        vpm = e.p_vpm.tile([128, 2, 256], F32R, tag="vpm")
        for jj in range(2):
            j = 2 * g + jj
            nc.vector.tensor_scalar_mul(
                vpm[:, jj, :], hpvp[:, jj, 256:512], bs.pmtile[:, j:j + 1]
            )
        st.vpms.append(vpm)
        nc.sync.dma_start(
            bs.opb[:, 2 * g:2 * g + 2, i * 512:i * 512 + 256],
            hpvp[:, :, 0:256],
        )
        yield

    # --- S scores -> At halves [128(c), 1024(p)] per (ck, n2) ---
    for ck in range(NCK):
        for n2 in range(2):
            ah = e.p_at.tile([128, 1024], F32R, tag="at")
            pss = e.ps_b.tile([128, 2, 512], F32, tag="psb")
            for sl in range(2):
                for k in range(2):
                    nc.tensor.matmul(
                        pss[:, sl, :],
                        st.ttT[:, k, ck * 128:(ck + 1) * 128],
                        bs.protT[:, k, n2 * 1024 + sl * 512:
                                n2 * 1024 + (sl + 1) * 512],
                        start=(k == 0), stop=(k == 1),
                    )
            nc.scalar.activation(ah[:, :], pss[:, :, :], TANH)
            st.ahalves[(ck, n2)] = ah
            yield


def _gen_back(e, bs, st):
    """S^T recompute + tanh, prot_cross per group, comp_cross chains."""
    nc = e.nc
    i = st.i
    for g in range(8):
        pst = e.ps_b.tile([128, 2, 512], F32, tag="psb")
        for jj in range(2):
            j = 2 * g + jj
            for k in range(2):
                nc.tensor.matmul(
                    pst[:, jj, :],
                    bs.protT[:, k, j * 128:(j + 1) * 128],
                    st.ttT[:, k, :],
                    start=(k == 0), stop=(k == 1),
                )
        atT = e.p_att.tile([128, 2, 512], F32R, tag="atT")
        nc.scalar.activation(atT[:, :, :], pst[:, :, :], TANH)
        st.atTs.append(atT)
        yield

        # prot_cross for p-chunks 2g, 2g+1
        n2 = g // 4
        pcp = e.ps_b.tile([128, 2, 256], F32, tag="psb")
        for jj in range(2):
            j = 2 * g + jj
            off = (j - n2 * 8) * 128
            for ck in range(NCK):
                nc.tensor.matmul(
                    pcp[:, jj, :],
                    st.ahalves[(ck, n2)][:, off:off + 128],
                    st.vcms[ck // 2][:, ck % 2, :],
                    start=(ck == 0), stop=(ck == NCK - 1),
                )
        pcr = e.p_out.tile([128, 2, 256], F32, tag="pcr", bufs=3)
        for jj in range(2):
            j = 2 * g + jj
            nc.vector.tensor_scalar_mul(
                pcr[:, jj, :], pcp[:, jj, :], bs.pmtile[:, j:j + 1]
            )
        nc.sync.dma_start(
            bs.opb[:, 2 * g:2 * g + 2, i * 512 + 256:i * 512 + 512],
            pcr[:, :, :],
        )
        if g == 3:
            # first halves of At no longer needed -> free slots for the
            # next layer's S units
            for ck in range(NCK):
                st.ahalves.pop((ck, 0))
        yield

    ccr = e.p_out.tile([128, NCK, 256], F32, tag="ccr")
    for ck in range(NCK):
        ccps = e.ps_cc.tile([128, 256], F32, tag="pscc", bufs=2)
        for g in range(8):
            for jj in range(2):
                nc.tensor.matmul(
                    ccps[:, :],
                    st.atTs[g][:, jj, ck * 128:(ck + 1) * 128],
                    st.vpms[g][:, jj, :],
                    start=(g == 0 and jj == 0),
                    stop=(g == 7 and jj == 1),
                )
        nc.vector.tensor_scalar_mul(
            ccr[:, ck, :], ccps[:, :], bs.cmtile[:, ck:ck + 1]
        )
        yield
    st.ahalves.clear()
    nc.sync.dma_start(bs.ocb[:, :, i * 512 + 256:i * 512 + 512], ccr[:, :, :])


def _drive(gens):
    """Round-robin the generators until all are exhausted."""
    gens = list(gens)
    while gens:
        for gen in list(gens):
            try:
                next(gen)
            except StopIteration:
                gens.remove(gen)


def _emit_all(e):
    bss = [_BState() for _ in range(BPC)]
    states = [[_LState(i) for i in range(L)] for _ in range(BPC)]
    for bi in range(BPC):
        bs, sts = bss[bi], states[bi]
        _drive([_gen_pre(e, bs, bi), _gen_front(e, bs, sts[0])])
        for i in range(L):
            gens = [_gen_back(e, bs, sts[i])]
            if i + 1 < L:
                gens.append(_gen_front(e, bs, sts[i + 1]))
            _drive(gens)


def _build_nc(with_proj_bias: bool):
    nc = bacc.Bacc("TRN2", target_bir_lowering=False, debug=False,
                   num_devices=NCORES)
    e = _Env()
    e.nc = nc
    e.with_bias = with_proj_bias

    e.comp = nc.dram_tensor("comp", [BPC, LC, D], F32, kind="ExternalInput")
    e.prot = nc.dram_tensor("prot", [BPC, LP, D], F32, kind="ExternalInput")
    e.cmt = nc.dram_tensor("cmt", [BPC, 128, NCK], F32, kind="ExternalInput")
    e.pmt = nc.dram_tensor("pmt", [BPC, 128, NPK], F32, kind="ExternalInput")
    e.usb = nc.dram_tensor("usb", [L, 128, 2, 2, 128], F32, kind="ExternalInput")
    e.wcd = nc.dram_tensor("wcd", [L, 128, 2, 512], F32, kind="ExternalInput")
    e.wpd = nc.dram_tensor("wpd", [L, 128, 2, 512], F32, kind="ExternalInput")
    e.bcd = nc.dram_tensor("bcd", [L, 1, 512], F32, kind="ExternalInput")
    e.onesd = nc.dram_tensor("onesd", [1, 1, 128], F32, kind="ExternalInput")
    e.bpd = nc.dram_tensor("bpd", [L, 1, 512], F32, kind="ExternalInput")

    e.oc = nc.dram_tensor("oc", [BPC, LC, 2 * D * L], F32, kind="ExternalOutput")
    e.op = nc.dram_tensor("op", [BPC, LP, 2 * D * L], F32, kind="ExternalOutput")

    with tile.TileContext(nc) as tc:
        with (
            tc.tile_pool(name="const", bufs=1) as p_const,
            tc.tile_pool(name="mask", bufs=2) as p_mask,
            tc.tile_pool(name="w", bufs=2) as p_w,
            tc.tile_pool(name="feat", bufs=1) as p_feat,
            tc.tile_pool(name="raw", bufs=1) as p_raw,
            tc.tile_pool(name="tt", bufs=(2 if with_proj_bias else 3)) as p_tt,
            tc.tile_pool(name="hv", bufs=2) as p_hv,
            tc.tile_pool(name="vm", bufs=4) as p_vm,
            tc.tile_pool(name="vpmp", bufs=(13 if with_proj_bias else 14)) as p_vpm,
            tc.tile_pool(name="atp", bufs=7) as p_at,
            tc.tile_pool(name="attp", bufs=8) as p_att,
            tc.tile_pool(name="outs", bufs=2) as p_out,
            tc.tile_pool(name="psb", bufs=3, space=bass.MemorySpace.PSUM) as ps_b,
        ):
            e.p_const, e.p_mask, e.p_w = p_const, p_mask, p_w
            e.p_feat, e.p_raw, e.p_tt = p_feat, p_raw, p_tt
            e.p_hv, e.p_vm, e.p_vpm = p_hv, p_vm, p_vpm
            e.p_at, e.p_att, e.p_out = p_at, p_att, p_out
            e.ps_b, e.ps_cc = ps_b, ps_b

            e.ident = p_const.tile([128, 128], F32, tag="ident")
            make_identity(nc, e.ident[:, :])
            if with_proj_bias:
                e.ones = p_const.tile([1, 128], F32R, tag="ones")
                nc.gpsimd.dma_start(e.ones[:, :], e.onesd[0])

            _emit_all(e)

    nc.compile()
    return nc


def _get_nc(with_proj_bias: bool):
    key = bool(with_proj_bias)
    if key not in _NC_CACHE:
        _NC_CACHE[key] = _build_nc(key)
    return _NC_CACHE[key]


def _masked_softmax_np(scores, mask):
    e = np.exp(scores - scores.max(axis=-1, keepdims=True)) * mask
    return e / (e.sum(axis=-1, keepdims=True) + 1e-6)


def kernel(comp_feat, comp_mask, prot_feat, prot_mask, U, W_p2c, b_p2c,
           W_c2p, b_c2p, W_hc, b_hc, W_hp, b_hp, W_ac, b_ac, W_ap, b_ap,
           W_cc, b_cc, W_cp, b_cp):
    comp_feat = np.ascontiguousarray(np.asarray(comp_feat, np.float32))
    prot_feat = np.ascontiguousarray(np.asarray(prot_feat, np.float32))
    cm = np.asarray(comp_mask).astype(np.float32)
    pm = np.asarray(prot_mask).astype(np.float32)
    U = np.asarray(U, np.float32)
    W_p2c = np.asarray(W_p2c, np.float32)
    W_c2p = np.asarray(W_c2p, np.float32)
    W_hc = np.asarray(W_hc, np.float32)
    W_hp = np.asarray(W_hp, np.float32)

    with_bias = bool(
        np.any(b_p2c) or np.any(b_c2p) or np.any(b_hc) or np.any(b_hp)
    )
    nc = _get_nc(with_bias)

    # host-side weight repacking into PE layouts
    usb = np.ascontiguousarray(
        U.reshape(L, 2, 128, 2, 128).transpose(0, 2, 1, 3, 4)
    )
    wcd = np.ascontiguousarray(
        np.concatenate([W_hc, W_c2p], axis=2).reshape(L, 2, 128, 512)
        .transpose(0, 2, 1, 3)
    )
    wpd = np.ascontiguousarray(
        np.concatenate([W_hp, W_p2c], axis=2).reshape(L, 2, 128, 512)
        .transpose(0, 2, 1, 3)
    )
    bcd = np.ascontiguousarray(
        np.concatenate([np.asarray(b_hc, np.float32),
                        np.asarray(b_c2p, np.float32)], axis=1).reshape(L, 1, 512)
    )
    bpd = np.ascontiguousarray(
        np.concatenate([np.asarray(b_hp, np.float32),
                        np.asarray(b_p2c, np.float32)], axis=1).reshape(L, 1, 512)
    )
    cmt = np.ascontiguousarray(cm.reshape(B, NCK, 128).transpose(0, 2, 1))
    pmt = np.ascontiguousarray(pm.reshape(B, NPK, 128).transpose(0, 2, 1))

    in_maps = []
    for c in range(NCORES):
        sl = slice(c * BPC, (c + 1) * BPC)
        in_maps.append({
            "comp": comp_feat[sl], "prot": prot_feat[sl],
            "cmt": cmt[sl], "pmt": pmt[sl],
            "usb": usb, "wcd": wcd, "wpd": wpd, "bcd": bcd, "bpd": bpd,
            "onesd": np.ones((1, 1, 128), np.float32),
        })

    res = run_bass_kernel_spmd(nc, in_maps, list(range(NCORES)))
    comp_layerwise = np.concatenate([res.results[c]["oc"] for c in range(NCORES)], 0)
    prot_layerwise = np.concatenate([res.results[c]["op"] for c in range(NCORES)], 0)

    # ---- host tail: attention pooling + final projections (fp32) ----
    W_ac = np.asarray(W_ac, np.float32)
    W_ap = np.asarray(W_ap, np.float32)
    comp_pools, prot_pools = [], []
    for i in range(L):
        comp_comb = comp_layerwise[:, :, i * 512:(i + 1) * 512]
        prot_comb = prot_layerwise[:, :, i * 512:(i + 1) * 512]
        sc = comp_comb @ W_ac[i] + np.asarray(b_ac, np.float32)[i]
        sp = prot_comb @ W_ap[i] + np.asarray(b_ap, np.float32)[i]
        cw = _masked_softmax_np(sc, cm)
        pw = _masked_softmax_np(sp, pm)
        comp_pools.append((comp_feat * cw[:, :, None]).sum(axis=1))
        prot_pools.append((prot_feat * pw[:, :, None]).sum(axis=1))
    comp_final = np.concatenate(comp_pools, axis=1) @ np.asarray(W_cc, np.float32) \
        + np.asarray(b_cc, np.float32)
    prot_final = np.concatenate(prot_pools, axis=1) @ np.asarray(W_cp, np.float32) \
        + np.asarray(b_cp, np.float32)

    return (comp_final.astype(np.float32), prot_final.astype(np.float32),
            comp_layerwise, prot_layerwise)


# revision 28
# speedup vs baseline: 1.0067x; 1.0067x over previous
"""Trainium2 Bass kernel for the BidirectionalAttentionModule problem.

Sharding: data-parallel over batch — 16 batches across 8 NeuronCores
(2 per core), weights replicated.

Device computes the two large outputs per batch b and layer i
(D=256, LC=512, LP=2048):
    Hc  = tanh(comp @ W_hc[i])                      [LC, D]
    Hp  = tanh(prot @ W_hp[i])                      [LP, D]
    At  = tanh(comp @ U[i] @ prot.T)                [LC, LP]   (unmasked)
    cc  = cm * (At  @ (pm * tanh(prot @ W_p2c[i]))) [LC, D]
    pc  = pm * (At.T @ (cm * tanh(comp @ W_c2p[i]))) [LP, D]
    comp_layerwise[b,:,512i:512i+512] = [Hc | cc]
    prot_layerwise[b,:,512i:512i+512] = [Hp | pc]
(the reference's masked A = At * cm[:,None] * pm[None,:] appears only
inside the two cross matmuls, so the mask folds into row/col scalings).

The softmax-pooling tail (comp_final / prot_final, <0.3% of FLOPs) is
computed on host in fp32 numpy from the returned layerwise tensors.

Matmuls run as float32r (single-pass reduced-precision fp32, ~1.2e-4
rel err) — 4x faster than exact fp32 on the PE.

The layer loop is software-pipelined: each layer's PE-dense back half
(S^T recompute + the two cross matmuls) is emitted interleaved with the
next layer's ACT-dense front half (projections + S scores) so neither
engine starves.
"""

import sys

if "/opt/trn_rl_repo" not in sys.path:
    sys.path.insert(0, "/opt/trn_rl_repo")

import numpy as np

import concourse.bass as bass
import concourse.mybir as mybir
import concourse.tile as tile
from concourse import bacc
from concourse.bass_utils import run_bass_kernel_spmd
from concourse.masks import make_identity

B, LC, LP, D, L = 16, 512, 2048, 256, 4
NCORES = 8
BPC = B // NCORES   # batches per core
NCK = LC // 128     # 4 comp chunks
NPK = LP // 128     # 16 prot chunks
F32 = mybir.dt.float32
F32R = mybir.dt.float32r
TANH = mybir.ActivationFunctionType.Tanh

_NC_CACHE = {}


class _Env:
    pass


class _LState:
    """Per-(batch, layer) tiles in flight."""
    def __init__(self, i):
        self.i = i
        self.ttT = None
        self.vcms = []
        self.vpms = []
        self.ahalves = {}   # (ck, n2) -> [128, 1024] tile of At rows ck
        self.atTs = []


class _BState:
    """Per-batch tiles/APs."""
    pass


def _gen_pre(e, bs, bi):
    """Feature loads + on-chip transposes for one batch, unit-at-a-time."""
    nc = e.nc
    # loads go out on the ACT HWDGE ring so they don't queue behind the
    # previous batch's store stream on the SP ring
    craw = e.p_raw.tile([128, NCK, D], F32, tag="craw")
    comp_r = e.comp[bi].rearrange("(c p) d -> p c d", p=128)
    nc.scalar.dma_start(craw[:, 0:2, :], comp_r[:, 0:2, :])
    nc.scalar.dma_start(craw[:, 2:4, :], comp_r[:, 2:4, :])
    prot_r = e.prot[bi].rearrange("(c p) d -> p c d", p=128)
    praws = []
    for p in range(4):
        pr = e.p_raw.tile([128, 4, D], F32, tag="praw", bufs=4, name=f"praw{p}")
        nc.scalar.dma_start(pr[:, :, :], prot_r[:, 4 * p:4 * p + 4, :])
        praws.append(pr)
    bs.cmtile = e.p_mask.tile([128, NCK], F32, tag="cmt")
    bs.pmtile = e.p_mask.tile([128, NPK], F32, tag="pmt")
    nc.scalar.dma_start(bs.cmtile[:, :], e.cmt[bi])
    nc.scalar.dma_start(bs.pmtile[:, :], e.pmt[bi])
    bs.ocb = e.oc[bi].rearrange("(c p) f -> p c f", p=128)   # [128, 4, 2048]
    bs.opb = e.op[bi].rearrange("(c p) f -> p c f", p=128)   # [128, 16, 2048]

    bs.compT = e.p_feat.tile([128, 2, LC], F32R, tag="compT")
    bs.protT = e.p_feat.tile([128, 2, LP], F32R, tag="protT")
    for k in range(2):
        psx = e.ps_b.tile([128, 2, 512], F32, tag="psb")
        for ck in range(NCK):
            nc.tensor.transpose(
                psx[:, 0, ck * 128:(ck + 1) * 128],
                craw[:, ck, k * 128:(k + 1) * 128],
                e.ident[:, :],
            )
        nc.vector.tensor_copy(bs.compT[:, k, :], psx[:, 0, :])
    yield
    for p in range(4):
        for k in range(2):
            psx = e.ps_b.tile([128, 2, 512], F32, tag="psb")
            for jj in range(4):
                nc.tensor.transpose(
                    psx[:, 0, jj * 128:(jj + 1) * 128],
                    praws[p][:, jj, k * 128:(k + 1) * 128],
                    e.ident[:, :],
                )
            nc.vector.tensor_copy(
                bs.protT[:, k, p * 512:(p + 1) * 512], psx[:, 0, :]
            )
        yield


def _gen_front(e, bs, st):
    """Weights+TT, comp projections, prot projections, S scores.
    Yields after each pipelineable unit."""
    nc = e.nc
    i = st.i

    # --- weights + T^T ---
    utile = e.p_w.tile([128, 2, 2, 128], F32R, tag="u")
    wct = e.p_w.tile([128, 2, 512], F32R, tag="wc")
    wpt = e.p_w.tile([128, 2, 512], F32R, tag="wp")
    nc.gpsimd.dma_start(utile[:, :, :, :], e.usb[i])
    nc.gpsimd.dma_start(wct[:, :, :], e.wcd[i])
    nc.gpsimd.dma_start(wpt[:, :, :], e.wpd[i])
    if e.with_bias:
        bct = e.p_w.tile([1, 512], F32R, tag="bc")
        bpt = e.p_w.tile([1, 512], F32R, tag="bp")
        nc.gpsimd.dma_start(bct[:, :], e.bcd[i])
        nc.gpsimd.dma_start(bpt[:, :], e.bpd[i])
    ttps = e.ps_b.tile([128, 2, 512], F32, tag="psb")
    for m in range(2):
        for k in range(2):
            nc.tensor.matmul(
                ttps[:, m, :], utile[:, k, m, :], bs.compT[:, k, :],
                start=(k == 0), stop=(k == 1),
            )
    st.ttT = e.p_tt.tile([128, 2, LC], F32R, tag="ttT")
    nc.vector.tensor_copy(st.ttT[:, :, :], ttps[:, :, :])
    yield

    # --- comp projections: [Hc | Vc] ---
    for cg in range(2):
        ppc = e.ps_b.tile([128, 2, 512], F32, tag="psb")
        for cc in range(2):
            ck = 2 * cg + cc
            if e.with_bias:
                nc.tensor.matmul(ppc[:, cc, :], e.ones[:, :], bct[:, :],
                                 start=True, stop=False)
            for k in range(2):
                nc.tensor.matmul(
                    ppc[:, cc, :],
                    bs.compT[:, k, ck * 128:(ck + 1) * 128],
                    wct[:, k, :],
                    start=(k == 0 and not e.with_bias), stop=(k == 1),
                )
        hcvc = e.p_hv.tile([128, 2, 512], F32, tag="hcvc", bufs=3)
        nc.scalar.activation(hcvc[:, :, :], ppc[:, :, :], TANH)
        vcm = e.p_vm.tile([128, 2, 256], F32R, tag="vcm")
        for cc in range(2):
            ck = 2 * cg + cc
            nc.vector.tensor_scalar_mul(
                vcm[:, cc, :], hcvc[:, cc, 256:512], bs.cmtile[:, ck:ck + 1]
            )
        st.vcms.append(vcm)
        nc.sync.dma_start(
            bs.ocb[:, 2 * cg:2 * cg + 2, i * 512:i * 512 + 256],
            hcvc[:, :, 0:256],
        )
        yield

    # --- prot projections: [Hp | Vp] ---
    for g in range(8):
        ppp = e.ps_b.tile([128, 2, 512], F32, tag="psb")
        for jj in range(2):
            j = 2 * g + jj
            if e.with_bias:
                nc.tensor.matmul(ppp[:, jj, :], e.ones[:, :], bpt[:, :],
                                 start=True, stop=False)
            for k in range(2):
                nc.tensor.matmul(
                    ppp[:, jj, :],
                    bs.protT[:, k, j * 128:(j + 1) * 128],
                    wpt[:, k, :],
                    start=(k == 0 and not e.with_bias), stop=(k == 1),
                )
        hpvp = e.p_hv.tile([128, 2, 512], F32, tag="hpvp", bufs=3)
        nc.scalar.activation(hpvp[:, :, :], ppp[:, :, :], TANH)
        vpm = e.p_vpm.tile([128, 2, 256], F32R, tag="vpm")
        for jj in range(2):
            j = 2 * g + jj
            nc.vector.tensor_scalar_mul(
                vpm[:, jj, :], hpvp[:, jj, 256:512], bs.pmtile[:, j:j + 1]
            )
        st.vpms.append(vpm)
        nc.sync.dma_start(
            bs.opb[:, 2 * g:2 * g + 2, i * 512:i * 512 + 256],
            hpvp[:, :, 0:256],
        )
        yield

    # --- S scores -> At halves [128(c), 1024(p)] per (ck, n2) ---
    for ck in range(NCK):
        for n2 in range(2):
            ah = e.p_at.tile([128, 1024], F32R, tag="at")
            pss = e.ps_b.tile([128, 2, 512], F32, tag="psb")
            for sl in range(2):
                for k in range(2):
                    nc.tensor.matmul(
                        pss[:, sl, :],
                        st.ttT[:, k, ck * 128:(ck + 1) * 128],
                        bs.protT[:, k, n2 * 1024 + sl * 512:
                                n2 * 1024 + (sl + 1) * 512],
                        start=(k == 0), stop=(k == 1),
                    )
            nc.scalar.activation(ah[:, :], pss[:, :, :], TANH)
            st.ahalves[(ck, n2)] = ah
            yield


def _gen_back(e, bs, st):
    """S^T recompute + tanh, prot_cross per group, comp_cross chains."""
    nc = e.nc
    i = st.i
    for g in range(8):
        pst = e.ps_b.tile([128, 2, 512], F32, tag="psb")
        for jj in range(2):
            j = 2 * g + jj
            for k in range(2):
                nc.tensor.matmul(
                    pst[:, jj, :],
                    bs.protT[:, k, j * 128:(j + 1) * 128],
                    st.ttT[:, k, :],
                    start=(k == 0), stop=(k == 1),
                )
        atT = e.p_att.tile([128, 2, 512], F32R, tag="atT")
        nc.scalar.activation(atT[:, :, :], pst[:, :, :], TANH)
        st.atTs.append(atT)
        yield

        # prot_cross for p-chunks 2g, 2g+1
        n2 = g // 4
        pcp = e.ps_b.tile([128, 2, 256], F32, tag="psb")
        for jj in range(2):
            j = 2 * g + jj
            off = (j - n2 * 8) * 128
            for ck in range(NCK):
                nc.tensor.matmul(
                    pcp[:, jj, :],
                    st.ahalves[(ck, n2)][:, off:off + 128],
                    st.vcms[ck // 2][:, ck % 2, :],
                    start=(ck == 0), stop=(ck == NCK - 1),
                )
        pcr = e.p_out.tile([128, 2, 256], F32, tag="pcr", bufs=3)
        for jj in range(2):
            j = 2 * g + jj
            nc.vector.tensor_scalar_mul(
                pcr[:, jj, :], pcp[:, jj, :], bs.pmtile[:, j:j + 1]
            )
        nc.sync.dma_start(
            bs.opb[:, 2 * g:2 * g + 2, i * 512 + 256:i * 512 + 512],
            pcr[:, :, :],
        )
        if g == 3:
            # first halves of At no longer needed -> free slots for the
            # next layer's S units
            for ck in range(NCK):
                st.ahalves.pop((ck, 0))
        yield

    ccr = e.p_out.tile([128, NCK, 256], F32, tag="ccr")
    for ck in range(NCK):
        ccps = e.ps_cc.tile([128, 256], F32, tag="pscc", bufs=2)
        for g in range(8):
            for jj in range(2):
                nc.tensor.matmul(
                    ccps[:, :],
                    st.atTs[g][:, jj, ck * 128:(ck + 1) * 128],
                    st.vpms[g][:, jj, :],
                    start=(g == 0 and jj == 0),
                    stop=(g == 7 and jj == 1),
                )
        nc.vector.tensor_scalar_mul(
            ccr[:, ck, :], ccps[:, :], bs.cmtile[:, ck:ck + 1]
        )
        yield
    st.ahalves.clear()
    nc.sync.dma_start(bs.ocb[:, :, i * 512 + 256:i * 512 + 512], ccr[:, :, :])


def _drive(gens):
    """Round-robin the generators until all are exhausted."""
    gens = list(gens)
    while gens:
        for gen in list(gens):
            try:
                next(gen)
            except StopIteration:
                gens.remove(gen)


def _emit_all(e):
    bss = [_BState() for _ in range(BPC)]
    states = [[_LState(i) for i in range(L)] for _ in range(BPC)]
    for bi in range(BPC):
        bs, sts = bss[bi], states[bi]
        _drive([_gen_pre(e, bs, bi), _gen_front(e, bs, sts[0])])
        for i in range(L):
            gens = [_gen_back(e, bs, sts[i])]
            if i + 1 < L:
                gens.append(_gen_front(e, bs, sts[i + 1]))
            _drive(gens)


def _build_nc(with_proj_bias: bool):
    nc = bacc.Bacc("TRN2", target_bir_lowering=False, debug=False,
                   num_devices=NCORES)
    e = _Env()
    e.nc = nc
    e.with_bias = with_proj_bias

    e.comp = nc.dram_tensor("comp", [BPC, LC, D], F32, kind="ExternalInput")
    e.prot = nc.dram_tensor("prot", [BPC, LP, D], F32, kind="ExternalInput")
    e.cmt = nc.dram_tensor("cmt", [BPC, 128, NCK], F32, kind="ExternalInput")
    e.pmt = nc.dram_tensor("pmt", [BPC, 128, NPK], F32, kind="ExternalInput")
    e.usb = nc.dram_tensor("usb", [L, 128, 2, 2, 128], F32, kind="ExternalInput")
    e.wcd = nc.dram_tensor("wcd", [L, 128, 2, 512], F32, kind="ExternalInput")
    e.wpd = nc.dram_tensor("wpd", [L, 128, 2, 512], F32, kind="ExternalInput")
    e.bcd = nc.dram_tensor("bcd", [L, 1, 512], F32, kind="ExternalInput")
    e.onesd = nc.dram_tensor("onesd", [1, 1, 128], F32, kind="ExternalInput")
    e.bpd = nc.dram_tensor("bpd", [L, 1, 512], F32, kind="ExternalInput")

    e.oc = nc.dram_tensor("oc", [BPC, LC, 2 * D * L], F32, kind="ExternalOutput")
    e.op = nc.dram_tensor("op", [BPC, LP, 2 * D * L], F32, kind="ExternalOutput")

    with tile.TileContext(nc) as tc:
        with (
            tc.tile_pool(name="const", bufs=1) as p_const,
            tc.tile_pool(name="mask", bufs=2) as p_mask,
            tc.tile_pool(name="w", bufs=2) as p_w,
            tc.tile_pool(name="feat", bufs=1) as p_feat,
            tc.tile_pool(name="raw", bufs=1) as p_raw,
            tc.tile_pool(name="tt", bufs=(2 if with_proj_bias else 3)) as p_tt,
            tc.tile_pool(name="hv", bufs=2) as p_hv,
            tc.tile_pool(name="vm", bufs=4) as p_vm,
            tc.tile_pool(name="vpmp", bufs=(13 if with_proj_bias else 14)) as p_vpm,
            tc.tile_pool(name="atp", bufs=7) as p_at,
            tc.tile_pool(name="attp", bufs=8) as p_att,
            tc.tile_pool(name="outs", bufs=2) as p_out,
            tc.tile_pool(name="psb", bufs=3, space=bass.MemorySpace.PSUM) as ps_b,
        ):
            e.p_const, e.p_mask, e.p_w = p_const, p_mask, p_w
            e.p_feat, e.p_raw, e.p_tt = p_feat, p_raw, p_tt
            e.p_hv, e.p_vm, e.p_vpm = p_hv, p_vm, p_vpm
            e.p_at, e.p_att, e.p_out = p_at, p_att, p_out
            e.ps_b, e.ps_cc = ps_b, ps_b

            e.ident = p_const.tile([128, 128], F32, tag="ident")
            make_identity(nc, e.ident[:, :])
            if with_proj_bias:
                e.ones = p_const.tile([1, 128], F32R, tag="ones")
                nc.gpsimd.dma_start(e.ones[:, :], e.onesd[0])

            _emit_all(e)

    nc.compile()
    return nc


def _get_nc(with_proj_bias: bool):
    key = bool(with_proj_bias)
    if key not in _NC_CACHE:
        _NC_CACHE[key] = _build_nc(key)
    return _NC_CACHE[key]


def _masked_softmax_np(scores, mask):
    e = np.exp(scores - scores.max(axis=-1, keepdims=True)) * mask
    return e / (e.sum(axis=-1, keepdims=True) + 1e-6)


def kernel(comp_feat, comp_mask, prot_feat, prot_mask, U, W_p2c, b_p2c,
           W_c2p, b_c2p, W_hc, b_hc, W_hp, b_hp, W_ac, b_ac, W_ap, b_ap,
           W_cc, b_cc, W_cp, b_cp):
    comp_feat = np.ascontiguousarray(np.asarray(comp_feat, np.float32))
    prot_feat = np.ascontiguousarray(np.asarray(prot_feat, np.float32))
    cm = np.asarray(comp_mask).astype(np.float32)
    pm = np.asarray(prot_mask).astype(np.float32)
    U = np.asarray(U, np.float32)
    W_p2c = np.asarray(W_p2c, np.float32)
    W_c2p = np.asarray(W_c2p, np.float32)
    W_hc = np.asarray(W_hc, np.float32)
    W_hp = np.asarray(W_hp, np.float32)

    with_bias = bool(
        np.any(b_p2c) or np.any(b_c2p) or np.any(b_hc) or np.any(b_hp)
    )
    nc = _get_nc(with_bias)

    # host-side weight repacking into PE layouts
    usb = np.ascontiguousarray(
        U.reshape(L, 2, 128, 2, 128).transpose(0, 2, 1, 3, 4)
    )
    wcd = np.ascontiguousarray(
        np.concatenate([W_hc, W_c2p], axis=2).reshape(L, 2, 128, 512)
        .transpose(0, 2, 1, 3)
    )
    wpd = np.ascontiguousarray(
        np.concatenate([W_hp, W_p2c], axis=2).reshape(L, 2, 128, 512)
        .transpose(0, 2, 1, 3)
    )
    bcd = np.ascontiguousarray(
        np.concatenate([np.asarray(b_hc, np.float32),
                        np.asarray(b_c2p, np.float32)], axis=1).reshape(L, 1, 512)
    )
    bpd = np.ascontiguousarray(
        np.concatenate([np.asarray(b_hp, np.float32),
                        np.asarray(b_p2c, np.float32)], axis=1).reshape(L, 1, 512)
    )
    cmt = np.ascontiguousarray(cm.reshape(B, NCK, 128).transpose(0, 2, 1))
    pmt = np.ascontiguousarray(pm.reshape(B, NPK, 128).transpose(0, 2, 1))

    in_maps = []
    for c in range(NCORES):
        sl = slice(c * BPC, (c + 1) * BPC)
        in_maps.append({
            "comp": comp_feat[sl], "prot": prot_feat[sl],
            "cmt": cmt[sl], "pmt": pmt[sl],
            "usb": usb, "wcd": wcd, "wpd": wpd, "bcd": bcd, "bpd": bpd,
            "onesd": np.ones((1, 1, 128), np.float32),
        })

    res = run_bass_kernel_spmd(nc, in_maps, list(range(NCORES)))
    comp_layerwise = np.concatenate([res.results[c]["oc"] for c in range(NCORES)], 0)
    prot_layerwise = np.concatenate([res.results[c]["op"] for c in range(NCORES)], 0)

    # ---- host tail: attention pooling + final projections (fp32) ----
    W_ac = np.asarray(W_ac, np.float32)
    W_ap = np.asarray(W_ap, np.float32)
    comp_pools, prot_pools = [], []
    for i in range(L):
        comp_comb = comp_layerwise[:, :, i * 512:(i + 1) * 512]
        prot_comb = prot_layerwise[:, :, i * 512:(i + 1) * 512]
        sc = comp_comb @ W_ac[i] + np.asarray(b_ac, np.float32)[i]
        sp = prot_comb @ W_ap[i] + np.asarray(b_ap, np.float32)[i]
        cw = _masked_softmax_np(sc, cm)
        pw = _masked_softmax_np(sp, pm)
        comp_pools.append((comp_feat * cw[:, :, None]).sum(axis=1))
        prot_pools.append((prot_feat * pw[:, :, None]).sum(axis=1))
    comp_final = np.concatenate(comp_pools, axis=1) @ np.asarray(W_cc, np.float32) \
        + np.asarray(b_cc, np.float32)
    prot_final = np.concatenate(prot_pools, axis=1) @ np.asarray(W_cp, np.float32) \
        + np.asarray(b_cp, np.float32)

    return (comp_final.astype(np.float32), prot_final.astype(np.float32),
            comp_layerwise, prot_layerwise)


# revision 30
# speedup vs baseline: 1.0525x; 1.0456x over previous
"""Trainium2 Bass kernel for the BidirectionalAttentionModule problem.

Sharding: data-parallel over batch — 16 batches across 8 NeuronCores
(2 per core), weights replicated.

Device computes the two large outputs per batch b and layer i
(D=256, LC=512, LP=2048):
    Hc  = tanh(comp @ W_hc[i])                      [LC, D]
    Hp  = tanh(prot @ W_hp[i])                      [LP, D]
    At  = tanh(comp @ U[i] @ prot.T)                [LC, LP]   (unmasked)
    cc  = cm * (At  @ (pm * tanh(prot @ W_p2c[i]))) [LC, D]
    pc  = pm * (At.T @ (cm * tanh(comp @ W_c2p[i]))) [LP, D]
    comp_layerwise[b,:,512i:512i+512] = [Hc | cc]
    prot_layerwise[b,:,512i:512i+512] = [Hp | pc]
(the reference's masked A = At * cm[:,None] * pm[None,:] appears only
inside the two cross matmuls, so the mask folds into row/col scalings).

The softmax-pooling tail (comp_final / prot_final, <0.3% of FLOPs) is
computed on host in fp32 numpy from the returned layerwise tensors.

Matmuls run as float32r (single-pass reduced-precision fp32, ~1.2e-4
rel err) — 4x faster than exact fp32 on the PE.

The layer loop is software-pipelined: each layer's PE-dense back half
(S^T recompute + the two cross matmuls) is emitted interleaved with the
next layer's ACT-dense front half (projections + S scores) so neither
engine starves.
"""

import sys

if "/opt/trn_rl_repo" not in sys.path:
    sys.path.insert(0, "/opt/trn_rl_repo")

import numpy as np

import concourse.bass as bass
import concourse.mybir as mybir
import concourse.tile as tile
from concourse import bacc
from concourse.bass_utils import run_bass_kernel_spmd
from concourse.masks import make_identity

B, LC, LP, D, L = 16, 512, 2048, 256, 4
NCORES = 8
BPC = B // NCORES   # batches per core
NCK = LC // 128     # 4 comp chunks
NPK = LP // 128     # 16 prot chunks
F32 = mybir.dt.float32
F32R = mybir.dt.float32r
TANH = mybir.ActivationFunctionType.Tanh

_NC_CACHE = {}


class _Env:
    pass


class _LState:
    """Per-(batch, layer) tiles in flight."""
    def __init__(self, i):
        self.i = i
        self.ttT = None
        self.vcms = []
        self.vpms = []
        self.ahalves = {}   # (ck, n2) -> [128, 1024] tile of At rows ck
        self.atTs = []


class _BState:
    """Per-batch tiles/APs."""
    pass


def _gen_pre(e, bs, bi):
    """Feature loads + on-chip transposes for one batch, unit-at-a-time."""
    nc = e.nc
    # loads go out on the ACT HWDGE ring so they don't queue behind the
    # previous batch's store stream on the SP ring
    craw = e.p_raw.tile([128, NCK, D], F32, tag="craw")
    comp_r = e.comp[bi].rearrange("(c p) d -> p c d", p=128)
    nc.scalar.dma_start(craw[:, 0:2, :], comp_r[:, 0:2, :])
    nc.scalar.dma_start(craw[:, 2:4, :], comp_r[:, 2:4, :])
    prot_r = e.prot[bi].rearrange("(c p) d -> p c d", p=128)
    praws = []
    for p in range(4):
        pr = e.p_raw.tile([128, 4, D], F32, tag="praw", bufs=4, name=f"praw{p}")
        nc.scalar.dma_start(pr[:, :, :], prot_r[:, 4 * p:4 * p + 4, :])
        praws.append(pr)
    bs.cmtile = e.p_mask.tile([128, NCK], F32, tag="cmt")
    bs.pmtile = e.p_mask.tile([128, NPK], F32, tag="pmt")
    nc.scalar.dma_start(bs.cmtile[:, :], e.cmt[bi])
    nc.scalar.dma_start(bs.pmtile[:, :], e.pmt[bi])
    bs.ocb = e.oc[bi].rearrange("(c p) f -> p c f", p=128)   # [128, 4, 2048]
    bs.opb = e.op[bi].rearrange("(c p) f -> p c f", p=128)   # [128, 16, 2048]

    bs.compT = e.p_feat.tile([128, 2, LC], F32R, tag="compT")
    bs.protT = e.p_feat.tile([128, 2, LP], F32R, tag="protT")
    for k in range(2):
        psx = e.ps_b.tile([128, 2, 512], F32, tag="psb")
        for ck in range(NCK):
            nc.tensor.transpose(
                psx[:, 0, ck * 128:(ck + 1) * 128],
                craw[:, ck, k * 128:(k + 1) * 128],
                e.ident[:, :],
            )
        nc.vector.tensor_copy(bs.compT[:, k, :], psx[:, 0, :])
    yield
    for p in range(4):
        for k in range(2):
            psx = e.ps_b.tile([128, 2, 512], F32, tag="psb")
            for jj in range(4):
                nc.tensor.transpose(
                    psx[:, 0, jj * 128:(jj + 1) * 128],
                    praws[p][:, jj, k * 128:(k + 1) * 128],
                    e.ident[:, :],
                )
            nc.vector.tensor_copy(
                bs.protT[:, k, p * 512:(p + 1) * 512], psx[:, 0, :]
            )
        yield


def _gen_front(e, bs, st):
    """Weights+TT, comp projections, prot projections, S scores.
    Yields after each pipelineable unit."""
    nc = e.nc
    i = st.i

    # --- weights + T^T ---
    utile = e.p_w.tile([128, 2, 2, 128], F32R, tag="u")
    wct = e.p_w.tile([128, 2, 512], F32R, tag="wc")
    wpt = e.p_w.tile([128, 2, 512], F32R, tag="wp")
    nc.gpsimd.dma_start(utile[:, :, :, :], e.usb[i])
    nc.gpsimd.dma_start(wct[:, :, :], e.wcd[i])
    nc.gpsimd.dma_start(wpt[:, :, :], e.wpd[i])
    if e.with_bias:
        bct = e.p_w.tile([1, 512], F32R, tag="bc")
        bpt = e.p_w.tile([1, 512], F32R, tag="bp")
        nc.gpsimd.dma_start(bct[:, :], e.bcd[i])
        nc.gpsimd.dma_start(bpt[:, :], e.bpd[i])
    ttps = e.ps_b.tile([128, 2, 512], F32, tag="psb")
    for m in range(2):
        for k in range(2):
            nc.tensor.matmul(
                ttps[:, m, :], utile[:, k, m, :], bs.compT[:, k, :],
                start=(k == 0), stop=(k == 1),
            )
    st.ttT = e.p_tt.tile([128, 2, LC], F32R, tag="ttT")
    nc.vector.tensor_copy(st.ttT[:, :, :], ttps[:, :, :])
    yield

    # --- comp projections: [Hc | Vc] ---
    for cg in range(2):
        ppc = e.ps_b.tile([128, 2, 512], F32, tag="psb")
        for cc in range(2):
            ck = 2 * cg + cc
            if e.with_bias:
                nc.tensor.matmul(ppc[:, cc, :], e.ones[:, :], bct[:, :],
                                 start=True, stop=False)
            for k in range(2):
                nc.tensor.matmul(
                    ppc[:, cc, :],
                    bs.compT[:, k, ck * 128:(ck + 1) * 128],
                    wct[:, k, :],
                    start=(k == 0 and not e.with_bias), stop=(k == 1),
                )
        hcvc = e.p_hv.tile([128, 2, 512], F32, tag="hcvc",
                           bufs=(2 if e.with_bias else 3))
        nc.scalar.activation(hcvc[:, :, :], ppc[:, :, :], TANH)
        vcm = e.p_vm.tile([128, 2, 256], F32R, tag="vcm")
        for cc in range(2):
            ck = 2 * cg + cc
            nc.vector.tensor_scalar_mul(
                vcm[:, cc, :], hcvc[:, cc, 256:512], bs.cmtile[:, ck:ck + 1]
            )
        st.vcms.append(vcm)
        nc.sync.dma_start(
            bs.ocb[:, 2 * cg:2 * cg + 2, i * 512:i * 512 + 256],
            hcvc[:, :, 0:256],
        )
        yield

    # --- prot projections: [Hp | Vp] ---
    for g in range(8):
        ppp = e.ps_b.tile([128, 2, 512], F32, tag="psb")
        for jj in range(2):
            j = 2 * g + jj
            if e.with_bias:
                nc.tensor.matmul(ppp[:, jj, :], e.ones[:, :], bpt[:, :],
                                 start=True, stop=False)
            for k in range(2):
                nc.tensor.matmul(
                    ppp[:, jj, :],
                    bs.protT[:, k, j * 128:(j + 1) * 128],
                    wpt[:, k, :],
                    start=(k == 0 and not e.with_bias), stop=(k == 1),
                )
        hpvp = e.p_hv.tile([128, 2, 512], F32, tag="hpvp", bufs=3)
        nc.scalar.activation(hpvp[:, :, :], ppp[:, :, :], TANH)
        vpm = e.p_vpm.tile([128, 2, 256], F32R, tag="vpm")
        for jj in range(2):
            j = 2 * g + jj
            nc.vector.tensor_scalar_mul(
                vpm[:, jj, :], hpvp[:, jj, 256:512], bs.pmtile[:, j:j + 1]
            )
        st.vpms.append(vpm)
        nc.sync.dma_start(
            bs.opb[:, 2 * g:2 * g + 2, i * 512:i * 512 + 256],
            hpvp[:, :, 0:256],
        )
        yield

    # --- S scores -> At halves [128(c), 1024(p)] per (ck, n2) ---
    for ck in range(NCK):
        for n2 in range(2):
            ah = e.p_at.tile([128, 1024], F32R, tag="at")
            pss = e.ps_b.tile([128, 2, 512], F32, tag="psb")
            for sl in range(2):
                for k in range(2):
                    nc.tensor.matmul(
                        pss[:, sl, :],
                        st.ttT[:, k, ck * 128:(ck + 1) * 128],
                        bs.protT[:, k, n2 * 1024 + sl * 512:
                                n2 * 1024 + (sl + 1) * 512],
                        start=(k == 0), stop=(k == 1),
                    )
            nc.scalar.activation(ah[:, :], pss[:, :, :], TANH)
            st.ahalves[(ck, n2)] = ah
            yield


def _gen_back(e, bs, st):
    """S^T recompute + tanh, prot_cross per group, comp_cross chains."""
    nc = e.nc
    i = st.i
    for g in range(8):
        pst = e.ps_b.tile([128, 2, 512], F32, tag="psb")
        for jj in range(2):
            j = 2 * g + jj
            for k in range(2):
                nc.tensor.matmul(
                    pst[:, jj, :],
                    bs.protT[:, k, j * 128:(j + 1) * 128],
                    st.ttT[:, k, :],
                    start=(k == 0), stop=(k == 1),
                )
        atT = e.p_att.tile([128, 2, 512], F32R, tag="atT")
        nc.scalar.activation(atT[:, :, :], pst[:, :, :], TANH)
        st.atTs.append(atT)
        yield

        # prot_cross for p-chunks 2g, 2g+1
        n2 = g // 4
        pcp = e.ps_cc.tile([128, 2, 256], F32, tag="pscc", bufs=2)
        for jj in range(2):
            j = 2 * g + jj
            off = (j - n2 * 8) * 128
            for ck in range(NCK):
                nc.tensor.matmul(
                    pcp[:, jj, :],
                    st.ahalves[(ck, n2)][:, off:off + 128],
                    st.vcms[ck // 2][:, ck % 2, :],
                    start=(ck == 0), stop=(ck == NCK - 1),
                )
        pcr = e.p_out.tile([128, 2, 256], F32, tag="pcr", bufs=3)
        for jj in range(2):
            j = 2 * g + jj
            nc.vector.tensor_scalar_mul(
                pcr[:, jj, :], pcp[:, jj, :], bs.pmtile[:, j:j + 1]
            )
        nc.sync.dma_start(
            bs.opb[:, 2 * g:2 * g + 2, i * 512 + 256:i * 512 + 512],
            pcr[:, :, :],
        )
        if g == 3:
            # first halves of At no longer needed -> free slots for the
            # next layer's S units
            for ck in range(NCK):
                st.ahalves.pop((ck, 0))
        yield

    ccr = e.p_out.tile([128, NCK, 256], F32, tag="ccr")
    for ck in range(NCK):
        ccps = e.ps_cc.tile([128, 256], F32, tag="pscc", bufs=2)
        for g in range(8):
            for jj in range(2):
                nc.tensor.matmul(
                    ccps[:, :],
                    st.atTs[g][:, jj, ck * 128:(ck + 1) * 128],
                    st.vpms[g][:, jj, :],
                    start=(g == 0 and jj == 0),
                    stop=(g == 7 and jj == 1),
                )
        nc.vector.tensor_scalar_mul(
            ccr[:, ck, :], ccps[:, :], bs.cmtile[:, ck:ck + 1]
        )
        yield
    st.ahalves.clear()
    nc.sync.dma_start(bs.ocb[:, :, i * 512 + 256:i * 512 + 512], ccr[:, :, :])


def _drive(gens):
    """Round-robin the generators until all are exhausted."""
    gens = list(gens)
    while gens:
        for gen in list(gens):
            try:
                next(gen)
            except StopIteration:
                gens.remove(gen)


def _emit_all(e):
    bss = [_BState() for _ in range(BPC)]
    states = [[_LState(i) for i in range(L)] for _ in range(BPC)]
    for bi in range(BPC):
        bs, sts = bss[bi], states[bi]
        _drive([_gen_pre(e, bs, bi), _gen_front(e, bs, sts[0])])
        for i in range(L):
            gens = [_gen_back(e, bs, sts[i])]
            if i + 1 < L:
                gens.append(_gen_front(e, bs, sts[i + 1]))
            _drive(gens)


def _build_nc(with_proj_bias: bool):
    nc = bacc.Bacc("TRN2", target_bir_lowering=False, debug=False,
                   num_devices=NCORES)
    e = _Env()
    e.nc = nc
    e.with_bias = with_proj_bias

    e.comp = nc.dram_tensor("comp", [BPC, LC, D], F32, kind="ExternalInput")
    e.prot = nc.dram_tensor("prot", [BPC, LP, D], F32, kind="ExternalInput")
    e.cmt = nc.dram_tensor("cmt", [BPC, 128, NCK], F32, kind="ExternalInput")
    e.pmt = nc.dram_tensor("pmt", [BPC, 128, NPK], F32, kind="ExternalInput")
    e.usb = nc.dram_tensor("usb", [L, 128, 2, 2, 128], F32, kind="ExternalInput")
    e.wcd = nc.dram_tensor("wcd", [L, 128, 2, 512], F32, kind="ExternalInput")
    e.wpd = nc.dram_tensor("wpd", [L, 128, 2, 512], F32, kind="ExternalInput")
    e.bcd = nc.dram_tensor("bcd", [L, 1, 512], F32, kind="ExternalInput")
    e.onesd = nc.dram_tensor("onesd", [1, 1, 128], F32, kind="ExternalInput")
    e.bpd = nc.dram_tensor("bpd", [L, 1, 512], F32, kind="ExternalInput")

    e.oc = nc.dram_tensor("oc", [BPC, LC, 2 * D * L], F32, kind="ExternalOutput")
    e.op = nc.dram_tensor("op", [BPC, LP, 2 * D * L], F32, kind="ExternalOutput")

    with tile.TileContext(nc) as tc:
        with (
            tc.tile_pool(name="const", bufs=1) as p_const,
            tc.tile_pool(name="mask", bufs=2) as p_mask,
            tc.tile_pool(name="w", bufs=2) as p_w,
            tc.tile_pool(name="feat", bufs=1) as p_feat,
            tc.tile_pool(name="raw", bufs=1) as p_raw,
            tc.tile_pool(name="tt", bufs=(2 if with_proj_bias else 3)) as p_tt,
            tc.tile_pool(name="hv", bufs=2) as p_hv,
            tc.tile_pool(name="vm", bufs=4) as p_vm,
            tc.tile_pool(name="vpmp", bufs=(13 if with_proj_bias else 14)) as p_vpm,
            tc.tile_pool(name="atp", bufs=7) as p_at,
            tc.tile_pool(name="attp", bufs=8) as p_att,
            tc.tile_pool(name="outs", bufs=2) as p_out,
            tc.tile_pool(name="psb", bufs=3, space=bass.MemorySpace.PSUM) as ps_b,
        ):
            e.p_const, e.p_mask, e.p_w = p_const, p_mask, p_w
            e.p_feat, e.p_raw, e.p_tt = p_feat, p_raw, p_tt
            e.p_hv, e.p_vm, e.p_vpm = p_hv, p_vm, p_vpm
            e.p_at, e.p_att, e.p_out = p_at, p_att, p_out
            e.ps_b, e.ps_cc = ps_b, ps_b

            e.ident = p_const.tile([128, 128], F32, tag="ident")
            make_identity(nc, e.ident[:, :])
            if with_proj_bias:
                e.ones = p_const.tile([1, 128], F32R, tag="ones")
                nc.gpsimd.dma_start(e.ones[:, :], e.onesd[0])

            _emit_all(e)

    nc.compile()
    return nc


def _get_nc(with_proj_bias: bool):
    key = bool(with_proj_bias)
    if key not in _NC_CACHE:
        _NC_CACHE[key] = _build_nc(key)
    return _NC_CACHE[key]


def _masked_softmax_np(scores, mask):
    e = np.exp(scores - scores.max(axis=-1, keepdims=True)) * mask
    return e / (e.sum(axis=-1, keepdims=True) + 1e-6)


def kernel(comp_feat, comp_mask, prot_feat, prot_mask, U, W_p2c, b_p2c,
           W_c2p, b_c2p, W_hc, b_hc, W_hp, b_hp, W_ac, b_ac, W_ap, b_ap,
           W_cc, b_cc, W_cp, b_cp):
    comp_feat = np.ascontiguousarray(np.asarray(comp_feat, np.float32))
    prot_feat = np.ascontiguousarray(np.asarray(prot_feat, np.float32))
    cm = np.asarray(comp_mask).astype(np.float32)
    pm = np.asarray(prot_mask).astype(np.float32)
    U = np.asarray(U, np.float32)
    W_p2c = np.asarray(W_p2c, np.float32)
    W_c2p = np.asarray(W_c2p, np.float32)
    W_hc = np.asarray(W_hc, np.float32)
    W_hp = np.asarray(W_hp, np.float32)

    with_bias = bool(
        np.any(b_p2c) or np.any(b_c2p) or np.any(b_hc) or np.any(b_hp)
    )
    nc = _get_nc(with_bias)

    # host-side weight repacking into PE layouts
    usb = np.ascontiguousarray(
        U.reshape(L, 2, 128, 2, 128).transpose(0, 2, 1, 3, 4)
    )
    wcd = np.ascontiguousarray(
        np.concatenate([W_hc, W_c2p], axis=2).reshape(L, 2, 128, 512)
        .transpose(0, 2, 1, 3)
    )
    wpd = np.ascontiguousarray(
        np.concatenate([W_hp, W_p2c], axis=2).reshape(L, 2, 128, 512)
        .transpose(0, 2, 1, 3)
    )
    bcd = np.ascontiguousarray(
        np.concatenate([np.asarray(b_hc, np.float32),
                        np.asarray(b_c2p, np.float32)], axis=1).reshape(L, 1, 512)
    )
    bpd = np.ascontiguousarray(
        np.concatenate([np.asarray(b_hp, np.float32),
                        np.asarray(b_p2c, np.float32)], axis=1).reshape(L, 1, 512)
    )
    cmt = np.ascontiguousarray(cm.reshape(B, NCK, 128).transpose(0, 2, 1))
    pmt = np.ascontiguousarray(pm.reshape(B, NPK, 128).transpose(0, 2, 1))

    in_maps = []
    for c in range(NCORES):
        sl = slice(c * BPC, (c + 1) * BPC)
        in_maps.append({
            "comp": comp_feat[sl], "prot": prot_feat[sl],
            "cmt": cmt[sl], "pmt": pmt[sl],
            "usb": usb, "wcd": wcd, "wpd": wpd, "bcd": bcd, "bpd": bpd,
            "onesd": np.ones((1, 1, 128), np.float32),
        })

    res = run_bass_kernel_spmd(nc, in_maps, list(range(NCORES)))
    comp_layerwise = np.concatenate([res.results[c]["oc"] for c in range(NCORES)], 0)
    prot_layerwise = np.concatenate([res.results[c]["op"] for c in range(NCORES)], 0)

    # ---- host tail: attention pooling + final projections (fp32) ----
    W_ac = np.asarray(W_ac, np.float32)
    W_ap = np.asarray(W_ap, np.float32)
    comp_pools, prot_pools = [], []
    for i in range(L):
        comp_comb = comp_layerwise[:, :, i * 512:(i + 1) * 512]
        prot_comb = prot_layerwise[:, :, i * 512:(i + 1) * 512]
        sc = comp_comb @ W_ac[i] + np.asarray(b_ac, np.float32)[i]
        sp = prot_comb @ W_ap[i] + np.asarray(b_ap, np.float32)[i]
        cw = _masked_softmax_np(sc, cm)
        pw = _masked_softmax_np(sp, pm)
        comp_pools.append((comp_feat * cw[:, :, None]).sum(axis=1))
        prot_pools.append((prot_feat * pw[:, :, None]).sum(axis=1))
    comp_final = np.concatenate(comp_pools, axis=1) @ np.asarray(W_cc, np.float32) \
        + np.asarray(b_cc, np.float32)
    prot_final = np.concatenate(prot_pools, axis=1) @ np.asarray(W_cp, np.float32) \
        + np.asarray(b_cp, np.float32)

    return (comp_final.astype(np.float32), prot_final.astype(np.float32),
            comp_layerwise, prot_layerwise)
